# revision 4
# baseline (speedup 1.0000x reference)
"""Int4-quantized column-parallel linear (LLaMA-7B FFN up-proj) on 8 TRN2 cores.

y[b,s,o] = sum_i x[b,s,i] * (unpack_int4(weight_q)[o,i] * scale[o]) + bias[o]

Strategy (per core, 1/8 of out_features = 1376):
  - fp8 DoubleRow matmul at 0.5 cycles/row (2x the fp16 rate). The two
    DoubleRow slots carry a hi/lo residual split of x: hi = fp8(x),
    lo = fp8(x - hi), so one DoubleRow matmul computes (hi+lo)^T @ w with
    ~2^-8 effective precision on x (rel err ~8e-4 end to end).
  - weights are int4 in [-8,7], exactly representable in fp8e4; the rhs AP
    duplicates the same weight bytes across both DoubleRow slots with a
    stride-0 dimension, so weights are stored once.
  - x hi/lo bytes are written interleaved (hi at even, lo at odd offsets) so
    one fp16-typed xbar DMA transpose moves both planes at once; the
    transposed pair block is exactly the [K, 2, tok] stationary AP DoubleRow
    wants.
  - 4-deep software pipeline: loads (SP+Pool queues) -> hi (ACT) -> lo (DVE)
    -> pair transposes (SP) -> matmuls (PE, 2 iterations behind) -> drain
    (DVE, 1 behind PE) -> store (Pool). PE is the only near-saturated engine.
"""

from contextlib import ExitStack

import numpy as np

import concourse.bass as bass
import concourse.tile as tile
from concourse import bacc, mybir
from concourse.masks import make_identity

F32 = mybir.dt.float32
F16 = mybir.dt.float16
F8 = mybir.dt.float8e4
I32 = mybir.dt.int32

B, S, IN, OUT = 4, 2048, 4096, 11008
NCORES = 8
TOK = B * S
FEAT = OUT // NCORES

P = 128


def _chunks(total, step):
    out = []
    c0 = 0
    while c0 < total:
        out.append((c0, min(step, total - c0)))
        c0 += step
    return out


def build(tok=TOK, in_dim=IN, feat=FEAT):
    assert tok % P == 0 and in_dim % 256 == 0
    kp = in_dim // P       # number of 128-deep K tiles
    ntok = tok // P        # number of 128-row token tiles
    half = in_dim // 2
    ftiles = _chunks(feat, P)      # phase-W feature tiles
    mchunks = _chunks(feat, 256)   # matmul output chunks (moving free = 512)
    KGRP = 16                      # transposes per PSUM staging tile
    n_tg = (kp + KGRP - 1) // KGRP

    nc = bacc.Bacc("TRN2", target_bir_lowering=False, debug=False,
                   num_devices=NCORES)
    x_d = nc.dram_tensor("x", [tok, in_dim], F32, kind="ExternalInput").ap()
    wq_d = nc.dram_tensor("wq", [feat, half], I32, kind="ExternalInput").ap()
    sc_d = nc.dram_tensor("scale", [feat], F32, kind="ExternalInput").ap()
    bi_d = nc.dram_tensor("bias", [feat], F32, kind="ExternalInput").ap()
    y_d = nc.dram_tensor("y", [tok, feat], F32, kind="ExternalOutput").ap()

    with tile.TileContext(nc) as tc, ExitStack() as ctx:
        const = ctx.enter_context(tc.tile_pool(name="const", bufs=1))
        wtp = ctx.enter_context(tc.tile_pool(name="wt", bufs=1))
        wscr = ctx.enter_context(tc.tile_pool(name="wscr", bufs=4))
        x32p = ctx.enter_context(tc.tile_pool(name="x32p", bufs=3))
        xpairp = ctx.enter_context(tc.tile_pool(name="xpairp", bufs=3))
        xtp = ctx.enter_context(tc.tile_pool(name="xtp", bufs=3))
        outp = ctx.enter_context(tc.tile_pool(name="outp", bufs=2))
        pstage = ctx.enter_context(tc.tile_pool(name="pstage", bufs=2, space="PSUM"))
        pout = ctx.enter_context(tc.tile_pool(name="pout", bufs=2, space="PSUM"))

        ident = const.tile([P, P], F8)
        make_identity(nc, ident[:])
        scale_b = const.tile([P, feat], F32)
        bias_b = const.tile([P, feat], F32)
        nc.sync.dma_start(
            out=scale_b[:],
            in_=bass.AP(tensor=sc_d.tensor, offset=sc_d.offset,
                        ap=[[0, P], sc_d.ap[0]]),
        )
        nc.sync.dma_start(
            out=bias_b[:],
            in_=bass.AP(tensor=bi_d.tensor, offset=bi_d.offset,
                        ap=[[0, P], bi_d.ap[0]]),
        )

        # Persistent fp8 weights, transposed: [in(part), k-major feat]
        wT = wtp.tile([P, kp * feat], F8)
        wTv = wT[:].rearrange("p (k f) -> p k f", k=kp)

        # ---- Phase W: unpack int4 -> fp8, transpose to [in, feat] ----
        def emit_phase_w():
            for f0, fsz in ftiles:
                wq_t = wscr.tile([P, half], I32, tag="w")
                nc.sync.dma_start(out=wq_t[:fsz], in_=wq_d[f0:f0 + fsz])
                wb = wscr.tile([P, in_dim], F8, tag="w")
                wbv = wb[:fsz].rearrange("p (i two) -> p two i", two=2)
                # sign-extend packed nibbles via shift pairs, convert to fp8
                nc.vector.tensor_scalar(
                    out=wbv[:, 0], in0=wq_t[:fsz], scalar1=28, scalar2=28,
                    op0=mybir.AluOpType.logical_shift_left,
                    op1=mybir.AluOpType.arith_shift_right)
                nc.vector.tensor_scalar(
                    out=wbv[:, 1], in0=wq_t[:fsz], scalar1=24, scalar2=28,
                    op0=mybir.AluOpType.logical_shift_left,
                    op1=mybir.AluOpType.arith_shift_right)
                for g in range(n_tg):
                    glen = min(KGRP, kp - g * KGRP)
                    st = pstage.tile([P, KGRP * P], F8)
                    for j in range(glen):
                        kb = g * KGRP + j
                        nc.tensor.transpose(
                            out=st[:, j * P:j * P + fsz],
                            in_=wb[:fsz, kb * P:(kb + 1) * P],
                            identity=ident[:fsz, :fsz])
                    stv = st[:].rearrange("p (j f) -> p j f", j=KGRP)
                    # stage copy on ACT (reads PSUM fine) so DVE stays free
                    nc.scalar.activation(
                        out=wTv[:, g * KGRP:g * KGRP + glen, f0:f0 + fsz],
                        in_=stv[:, :glen, :fsz],
                        func=mybir.ActivationFunctionType.Copy)

        emit_phase_w()

        # ---- Main loop: 4-deep software pipeline over token tiles ----
        x32s, xpairs, xts, pos, ots = {}, {}, {}, {}, {}

        def emit_load(i):
            x32 = x32p.tile([P, in_dim], F32)
            x32s[i] = x32
            nc.gpsimd.dma_start(out=x32[:, :half],
                                in_=x_d[i * P:(i + 1) * P, :half])
            nc.gpsimd.dma_start(out=x32[:, half:],
                                in_=x_d[i * P:(i + 1) * P, half:])

        def emit_hi(i):
            x32 = x32s[i]
            xpair = xpairp.tile([P, 2 * in_dim], F8)
            xpairs[i] = xpair
            xpv = xpair[:].rearrange("p (i two) -> p two i", two=2)
            nc.scalar.activation(out=xpv[:, 0], in_=x32[:],
                                 func=mybir.ActivationFunctionType.Copy)

        def emit_lo(i):
            x32 = x32s[i]
            xpv = xpairs[i][:].rearrange("p (i two) -> p two i", two=2)
            nc.vector.tensor_tensor(out=xpv[:, 1], in0=x32[:], in1=xpv[:, 0],
                                    op=mybir.AluOpType.subtract)

        def emit_transposes(i):
            xpair = xpairs[i]
            xt = xtp.tile([P, kp * 2 * P], F8)
            xts[i] = xt
            for k in range(kp):
                nc.sync.dma_start_transpose(
                    out=xt[:, k * 2 * P:(k + 1) * 2 * P].bitcast(F16),
                    in_=xpair[:, k * 2 * P:(k + 1) * 2 * P].bitcast(F16))
            del xpairs[i]

        def emit_matmuls(i):
            xt = xts[i]
            po = pout.tile([P, feat], F32)
            pos[i] = po
            for k in range(kp):
                lhsT = bass.AP(
                    tensor=xt.tensor, offset=xt[:].offset + k * 2 * P,
                    ap=[xt[:].ap[0], [1, 2], [2, P]])
                for ci, (c0, csz) in enumerate(mchunks):
                    first_in_bank = c0 % 512 == 0
                    last_in_bank = (ci == len(mchunks) - 1
                                    or mchunks[ci + 1][0] % 512 == 0)
                    rhs = bass.AP(
                        tensor=wT.tensor, offset=wT[:].offset + k * feat + c0,
                        ap=[wT[:].ap[0], [0, 2], [1, csz]])
                    nc.tensor.matmul(
                        out=po[:, c0:c0 + csz], lhsT=lhsT, rhs=rhs,
                        start=(k == 0 and first_in_bank),
                        stop=(k == kp - 1 and last_in_bank),
                        perf_mode=mybir.MatmulPerfMode.DoubleRow)
            del x32s[i], xts[i]

        def emit_drain(i):
            po = pos[i]
            ot = outp.tile([P, feat], F32)
            ots[i] = ot
            nc.vector.tensor_tensor(out=ot[:], in0=po[:], in1=scale_b[:],
                                    op=mybir.AluOpType.mult)
            nc.vector.tensor_tensor(out=ot[:], in0=ot[:], in1=bias_b[:],
                                    op=mybir.AluOpType.add)
            del pos[i]

        def emit_store(i):
            nc.sync.dma_start(out=y_d[i * P:(i + 1) * P, :], in_=ots[i][:])
            del ots[i]

        for i in range(ntok + 4):
            if i < ntok:
                emit_load(i)
            if 1 <= i <= ntok:
                emit_hi(i - 1)
                emit_lo(i - 1)
            if 4 <= i:
                emit_drain(i - 4)
            if 1 <= i <= ntok:
                emit_transposes(i - 1)
            if 3 <= i < ntok + 3:
                emit_matmuls(i - 3)
            if 4 <= i:
                emit_store(i - 4)

    nc.compile()
    return nc


_CACHE = {}


def _get_program():
    if "nc" not in _CACHE:
        _CACHE["nc"] = build()
    return _CACHE["nc"]


def kernel(x, weight_q, scale, bias):
    from concourse.bass_utils import run_bass_kernel_spmd

    try:
        import jax

        jax.config.update("jax_compilation_cache_dir", "/root/problem/jax_cache")
        jax.config.update("jax_persistent_cache_min_compile_time_secs", 0)
    except Exception:
        pass

    nc = _get_program()
    xr = np.ascontiguousarray(np.asarray(x, dtype=np.float32).reshape(TOK, IN))
    wq = np.asarray(weight_q, dtype=np.int32)
    sc = np.asarray(scale, dtype=np.float32)
    bi = np.asarray(bias, dtype=np.float32)
    in_maps = []
    for c in range(NCORES):
        f0 = c * FEAT
        in_maps.append({
            "x": xr,
            "wq": np.ascontiguousarray(wq[f0:f0 + FEAT]),
            "scale": np.ascontiguousarray(sc[f0:f0 + FEAT]),
            "bias": np.ascontiguousarray(bi[f0:f0 + FEAT]),
        })
    res = run_bass_kernel_spmd(nc, in_maps, list(range(NCORES))).results
    y = np.concatenate([res[c]["y"] for c in range(NCORES)], axis=1)
    return y.reshape(B, S, OUT)
